# revision 5
# baseline (speedup 1.0000x reference)
"""Trainium2 Bass kernel for nn_Encoder (dense MLP with stochastic ternarization).

y = tanh(x @ (s1*T(w1,n1)) + b1) @ (s2*T(w2,n2)) + b2,  T(w,n) = (w-n>1) - (w-n<-1)

Strategy: tensor-parallel over the 16384 hidden dim across 8 cores.
Each core gets a 2048-wide hidden shard of w1/noise1/s1/b1 (column-sharded) and
the matching 2048-row shard of w2/noise2. x is replicated (host pre-transposed
to bf16 xT so the contraction dim lands on SBUF partitions). Each core computes
a partial yT = (h_shard @ w2_shard).T in fp32; a ReduceScatter(add) over the 8
cores then hands core c the summed rows 128c:128(c+1) of yT, where s2/b2 are
applied. The host concatenates the 8 shards and transposes back.

Ternarization on device: q = w - noise (DVE), then tanh(2^30*(q-1)) +
tanh(2^30*(q+1)) (ACT) which is exactly (q>1)-(q<-1) doubled, i.e. 2*T; the
extra factor 2 is folded into s1/s2 on the host (passed as 0.5*s).
"""

import sys

for _p in ("/opt/trn_rl_repo",):
    if _p not in sys.path:
        sys.path.insert(0, _p)

import numpy as np
import ml_dtypes

import concourse.bass as bass
import concourse.bacc as bacc
import concourse.mybir as mybir
import concourse.tile as tile
from concourse.bass_utils import run_bass_kernel_spmd

BF16 = mybir.dt.bfloat16
F32 = mybir.dt.float32
NPBF16 = ml_dtypes.bfloat16

N_CORES = 8
B = 2048          # batch
DIN = 3072        # input dim
DHID = 16384      # hidden dim
DOUT = 1024       # output dim
HSH = DHID // N_CORES   # 2048 hidden per core
DSH = DOUT // N_CORES   # 128 output rows per core after reduce-scatter

K1 = DIN // 128          # 24 contraction tiles, layer 1
K2 = HSH // 128          # 16 contraction tiles, layer 2
NB = B // 512            # 4 batch blocks of 512
NG = 4                   # hidden-shard groups (512 wide each)
GW = HSH // NG           # 512 group width
MG = GW // 128           # 4 output tiles of 128 per group
ND = DOUT // 128         # 8 dout tiles

BIGK = float(2 ** 30)    # tanh(BIGK*(q -+ 1)) == sign(q -+ 1) exactly in fp32

TANH = mybir.ActivationFunctionType.Tanh
IDENT = mybir.ActivationFunctionType.Identity


def build_bass():
    nc = bacc.Bacc("TRN2", target_bir_lowering=False, debug=False, num_devices=N_CORES)

    xT = nc.dram_tensor("xT", [DIN, B], BF16, kind="ExternalInput")
    w1s = nc.dram_tensor("w1s", [DIN, HSH], F32, kind="ExternalInput")
    n1s = nc.dram_tensor("n1s", [DIN, HSH], F32, kind="ExternalInput")
    s1h = nc.dram_tensor("s1h", [128, DHID // 128 // N_CORES], F32, kind="ExternalInput")
    b1m = nc.dram_tensor("b1m", [128, DHID // 128 // N_CORES], F32, kind="ExternalInput")
    w2s = nc.dram_tensor("w2s", [HSH, DOUT], F32, kind="ExternalInput")
    n2s = nc.dram_tensor("n2s", [HSH, DOUT], F32, kind="ExternalInput")
    s2c = nc.dram_tensor("s2c", [128, 1], F32, kind="ExternalInput")
    b2c = nc.dram_tensor("b2c", [128, 1], F32, kind="ExternalInput")

    yTc = nc.dram_tensor("yTc", [DSH, B], F32, kind="ExternalOutput")

    with tile.TileContext(nc) as tc:
        with (
            tc.tile_pool(name="const", bufs=1) as cpool,
            tc.tile_pool(name="dram", bufs=1, space="DRAM") as dpool,
        ):
            s1_sb = cpool.tile([128, K2], F32, tag="s1")
            b1_sb = cpool.tile([128, K2], F32, tag="b1")
            s2_sb = cpool.tile([128, 1], F32, tag="s2")
            b2_sb = cpool.tile([128, 1], F32, tag="b2")
            nc.sync.dma_start(s1_sb[:], s1h[:, :])
            nc.sync.dma_start(b1_sb[:], b1m[:, :])
            nc.sync.dma_start(s2_sb[:], s2c[:, :])
            nc.sync.dma_start(b2_sb[:], b2c[:, :])
            kneg = cpool.tile([128, 1], F32, tag="kneg")
            nc.vector.memset(kneg[:], -BIGK)
            kpos = cpool.tile([128, 1], F32, tag="kpos")
            nc.vector.memset(kpos[:], BIGK)

            hT_d = dpool.tile([HSH, B], BF16, tag="hT")
            yT_part = dpool.tile([DOUT, B], F32, tag="yTp")
            rs_out = dpool.tile([DSH, B], F32, tag="rs")

            # ---------------- phase A+B: x load, layer-1 ----------------
            with tc.tile_pool(name="xt", bufs=1) as xpool:
                xt = xpool.tile([128, K1, B], BF16, tag="xt")
                for k in range(K1):
                    nc.sync.dma_start(xt[:, k, :], xT[k * 128:(k + 1) * 128, :])

                with (
                    tc.tile_pool(name="l1stage", bufs=2) as wpool,
                    tc.tile_pool(name="t2", bufs=2) as tpool,
                    tc.tile_pool(name="ps1", bufs=4, space="PSUM") as pspool,
                    tc.tile_pool(name="hout", bufs=4) as hpool,
                ):
                    for g in range(NG):
                        t2 = tpool.tile([128, K1, GW], BF16, tag="t2")
                        for k in range(K1):
                            w_t = wpool.tile([128, GW], F32, tag="w")
                            nc.sync.dma_start(
                                w_t[:], w1s[k * 128:(k + 1) * 128, g * GW:(g + 1) * GW]
                            )
                            n_t = wpool.tile([128, GW], F32, tag="n")
                            nc.sync.dma_start(
                                n_t[:], n1s[k * 128:(k + 1) * 128, g * GW:(g + 1) * GW]
                            )
                            q_t = wpool.tile([128, GW], F32, tag="q")
                            nc.vector.tensor_sub(q_t[:], w_t[:], n_t[:])
                            a1 = wpool.tile([128, GW], BF16, tag="a1")
                            nc.scalar.activation(a1[:], q_t[:], TANH, bias=kneg[:, 0:1], scale=BIGK)
                            a2 = wpool.tile([128, GW], BF16, tag="a2")
                            nc.scalar.activation(a2[:], q_t[:], TANH, bias=kpos[:, 0:1], scale=BIGK)
                            nc.vector.tensor_add(t2[:, k, :], a1[:], a2[:])

                        for m in range(MG):
                            mabs = g * MG + m
                            for n in range(NB):
                                ps = pspool.tile([128, 512], F32, tag="ps")
                                for k in range(K1):
                                    nc.tensor.matmul(
                                        ps[:],
                                        t2[:, k, m * 128:(m + 1) * 128],
                                        xt[:, k, n * 512:(n + 1) * 512],
                                        start=(k == 0),
                                        stop=(k == K1 - 1),
                                    )
                                h_sb = hpool.tile([128, 512], BF16, tag="h")
                                nc.scalar.activation(
                                    h_sb[:], ps[:], TANH,
                                    bias=b1_sb[:, mabs:mabs + 1],
                                    scale=s1_sb[:, mabs:mabs + 1],
                                )
                                nc.sync.dma_start(
                                    hT_d[mabs * 128:(mabs + 1) * 128, n * 512:(n + 1) * 512],
                                    h_sb[:],
                                )

            # ---------------- phase C: layer-2 ----------------
            with (
                tc.tile_pool(name="l2stage", bufs=2) as wpool2,
                tc.tile_pool(name="t22", bufs=1) as tpool2,
                tc.tile_pool(name="hread", bufs=2) as hrpool,
                tc.tile_pool(name="ps2", bufs=4, space="PSUM") as ps2pool,
                tc.tile_pool(name="yout", bufs=4) as ypool,
            ):
                t22 = tpool2.tile([128, K2, DOUT], BF16, tag="t22")
                for k2 in range(K2):
                    w_t = wpool2.tile([128, DOUT], F32, tag="w2")
                    nc.sync.dma_start(w_t[:], w2s[k2 * 128:(k2 + 1) * 128, :])
                    n_t = wpool2.tile([128, DOUT], F32, tag="n2")
                    nc.sync.dma_start(n_t[:], n2s[k2 * 128:(k2 + 1) * 128, :])
                    q_t = wpool2.tile([128, DOUT], F32, tag="q2")
                    nc.vector.tensor_sub(q_t[:], w_t[:], n_t[:])
                    a1 = wpool2.tile([128, DOUT], BF16, tag="a12")
                    nc.scalar.activation(a1[:], q_t[:], TANH, bias=kneg[:, 0:1], scale=BIGK)
                    a2 = wpool2.tile([128, DOUT], BF16, tag="a22")
                    nc.scalar.activation(a2[:], q_t[:], TANH, bias=kpos[:, 0:1], scale=BIGK)
                    nc.vector.tensor_add(t22[:, k2, :], a1[:], a2[:])

                for n in range(NB):
                    hld = hrpool.tile([128, K2, 512], BF16, tag="hl")
                    for k2 in range(K2):
                        nc.sync.dma_start(
                            hld[:, k2, :],
                            hT_d[k2 * 128:(k2 + 1) * 128, n * 512:(n + 1) * 512],
                        )
                    for d in range(ND):
                        ps = ps2pool.tile([128, 512], F32, tag="ps2")
                        for k2 in range(K2):
                            nc.tensor.matmul(
                                ps[:],
                                t22[:, k2, d * 128:(d + 1) * 128],
                                hld[:, k2, :],
                                start=(k2 == 0),
                                stop=(k2 == K2 - 1),
                            )
                        y_sb = ypool.tile([128, 512], F32, tag="y")
                        nc.scalar.copy(y_sb[:], ps[:])
                        nc.sync.dma_start(
                            yT_part[d * 128:(d + 1) * 128, n * 512:(n + 1) * 512],
                            y_sb[:],
                        )

            # ---------------- phase D+E: reduce-scatter, scale/bias, out ----------------
            nc.gpsimd.collective_compute(
                "ReduceScatter",
                mybir.AluOpType.add,
                replica_groups=[list(range(N_CORES))],
                ins=[yT_part.opt()],
                outs=[rs_out.opt()],
            )
            with tc.tile_pool(name="fin", bufs=1) as fpool:
                rs_sb = fpool.tile([128, B], F32, tag="rsb")
                nc.sync.dma_start(rs_sb[:], rs_out[:, :])
                out_sb = fpool.tile([128, B], F32, tag="osb")
                nc.scalar.activation(
                    out_sb[:], rs_sb[:], IDENT,
                    bias=b2_sb[:, 0:1], scale=s2_sb[:, 0:1],
                )
                nc.sync.dma_start(yTc[:, :], out_sb[:])

    nc.compile()
    return nc


_NC_CACHE = {}


def _get_nc():
    if "nc" not in _NC_CACHE:
        _NC_CACHE["nc"] = build_bass()
    return _NC_CACHE["nc"]


def _make_in_maps(x, w1, s1, b1, w2, s2, b2, noise1, noise2):
    x = np.asarray(x, dtype=np.float32)
    w1 = np.asarray(w1, dtype=np.float32)
    s1 = np.asarray(s1, dtype=np.float32)
    b1 = np.asarray(b1, dtype=np.float32)
    w2 = np.asarray(w2, dtype=np.float32)
    s2 = np.asarray(s2, dtype=np.float32)
    b2 = np.asarray(b2, dtype=np.float32)
    noise1 = np.asarray(noise1, dtype=np.float32)
    noise2 = np.asarray(noise2, dtype=np.float32)

    xT = np.ascontiguousarray(x.T).astype(NPBF16)
    in_maps = []
    for c in range(N_CORES):
        hs = slice(c * HSH, (c + 1) * HSH)
        ds = slice(c * DSH, (c + 1) * DSH)
        in_maps.append({
            "xT": xT,
            "w1s": np.ascontiguousarray(w1[:, hs]),
            "n1s": np.ascontiguousarray(noise1[:, hs]),
            "s1h": np.ascontiguousarray((0.5 * s1[hs]).reshape(K2, 128).T),
            "b1m": np.ascontiguousarray(b1[hs].reshape(K2, 128).T),
            "w2s": np.ascontiguousarray(w2[hs, :]),
            "n2s": np.ascontiguousarray(noise2[hs, :]),
            "s2c": np.ascontiguousarray((0.5 * s2[ds]).reshape(128, 1)),
            "b2c": np.ascontiguousarray(b2[ds].reshape(128, 1)),
        })
    return in_maps


def kernel(x, w1, s1, b1, w2, s2, b2, noise1, noise2, _bench_out=None):
    """Full-input, full-output entry point. Shards across 8 NeuronCores."""
    nc = _get_nc()
    in_maps = _make_in_maps(x, w1, s1, b1, w2, s2, b2, noise1, noise2)
    res = run_bass_kernel_spmd(nc, in_maps, core_ids=list(range(N_CORES)))
    if _bench_out is not None:
        _bench_out.append(res)
    yT = np.concatenate([res.results[c]["yTc"] for c in range(N_CORES)], axis=0)
    return np.ascontiguousarray(yT.T).astype(np.float32)


if __name__ == "__main__":
    # smoke build
    nc = build_bass()
    print("built OK")
